# revision 1
# baseline (speedup 1.0000x reference)
"""Trainium2 Bass kernel for nn_Net_89361089561102 (2-layer dense transformer,
NF4-quantized weights, cls head). Tensor-parallel over 8 NeuronCores.

Strategy:
 - Host: unpack NF4 weights -> bf16, pre-transpose to [K, M] layout, shard
   TP-style (qkv/gate_up by output dim == heads/ff, o/down by output dim),
   embedding gather, RoPE cos/sin tables, causal masks.
 - Device (per core, feature-major activations [H partitions, tokens free]):
   rmsnorm (PE ones-matmul partition reductions), qkv projections, RoPE,
   attention with transposed scores [tk, tq] (softmax denominator via PE),
   AllGather(ctx) -> o_proj (output-sharded) -> AllGather(x), gated MLP with
   AllGather(intermediate) -> down (output-sharded) -> AllGather(x).
 - Layer 2 (last layer): q/o/MLP evaluated only at the last token of each
   batch (the only positions that reach the output); k/v still full.
 - Final rmsnorm + cls head (Linear-ReLU-LayerNorm-Linear) computed
   redundantly on every core for the 2 last tokens.
"""

import math
from contextlib import ExitStack
from dataclasses import dataclass

import numpy as np
import ml_dtypes

BF16 = ml_dtypes.bfloat16
EPS = 1e-5
BLK = 64
NF4 = np.array([
    -1.0, -0.6961928009986877, -0.5250730514526367, -0.39491748809814453,
    -0.28444138169288635, -0.18477343022823334, -0.09105003625154495, 0.0,
    0.07958029955625534, 0.16093020141124725, 0.24611230194568634,
    0.33791524171829224, 0.44070982933044434, 0.5626170039176941,
    0.7229568362236023, 1.0], dtype=np.float32)


@dataclass(frozen=True)
class Cfg:
    H: int
    NH: int
    HD: int
    FF: int
    B: int
    S: int
    L: int
    NC: int
    CLS: int = 768
    NCLS: int = 2
    P: int = 128

    @property
    def T(self):
        return self.B * self.S

    @property
    def KT(self):
        return self.H // self.P

    @property
    def KTF(self):
        return self.FF // self.P

    @property
    def HPC(self):  # heads per core
        return self.NH // self.NC

    @property
    def DR(self):  # q/k/v rows per core
        return self.HPC * self.HD

    @property
    def OR(self):  # o/down output rows per core
        return self.H // self.NC

    @property
    def OT(self):
        return self.OR // self.P

    @property
    def FPC(self):  # ff rows per core
        return self.FF // self.NC

    @property
    def FT(self):
        return self.FPC // self.P

    @property
    def SP(self):  # seq tiles per batch
        return self.S // self.P

    @property
    def TP_(self):  # token tiles total
        return self.T // self.P

    @property
    def CT(self):
        return self.CLS // self.P

    def check(self):
        assert self.H % self.P == 0 and self.FF % self.P == 0
        assert self.S % self.P == 0 and self.S <= 512
        assert self.NH % self.NC == 0 and self.H % self.NC == 0
        assert self.FF % self.NC == 0
        assert self.OR % self.P == 0 and self.FPC % self.P == 0
        assert self.HD <= self.P and self.HD % 2 == 0
        assert self.HPC * self.B <= 8  # q/k psum groups
        assert self.TP_ <= 8           # v psum groups
        assert self.OT * self.B <= 8   # o/down psum groups
        assert self.CLS % self.P == 0


FULL_CFG = Cfg(H=3072, NH=32, HD=96, FF=8192, B=2, S=512, L=2, NC=8)


# ----------------------------------------------------------------------------
# host-side prep
# ----------------------------------------------------------------------------

def dequant_np(packed, absmax, out_f, in_f):
    shifts = (np.arange(8, dtype=np.int32) * 4)
    codes = ((packed[:, None] >> shifts) & 0xF).reshape(-1)
    w = (NF4[codes].reshape(-1, BLK) * absmax[:, None].astype(np.float32))
    return w.reshape(out_f, in_f)


def _wt3(w_t, P):
    """[K, M] fp32 -> [K//P, P, M] bf16 contiguous."""
    K, M = w_t.shape
    return np.ascontiguousarray(w_t.reshape(K // P, P, M).astype(BF16))


def host_prep(cfg: Cfg, inputs):
    """Full inputs -> list of per-core input maps."""
    c = cfg
    P = c.P
    x = inputs["embed"][inputs["input_ids"]]          # [B, S, H] fp32
    x0f = np.ascontiguousarray(x.reshape(c.T, c.H).T.astype(np.float32))
    x0 = np.ascontiguousarray(x0f.astype(BF16))        # [H, T] bf16

    # rope tables
    inv = 1.0 / (10000.0 ** (np.arange(0, c.HD, 2, dtype=np.float32) / c.HD))
    f = np.outer(np.arange(c.S, dtype=np.float32), inv)
    emb = np.concatenate([f, f], -1)                   # [S, HD]
    sgn = np.concatenate([-np.ones(c.HD // 2, np.float32),
                          np.ones(c.HD // 2, np.float32)])
    cosT = np.tile(np.cos(emb).T, (1, c.B))            # [HD, T]
    sinT = np.tile(np.sin(emb).T * sgn[:, None], (1, c.B))
    last = np.array([b * c.S + c.S - 1 for b in range(c.B)])
    cosT2 = np.ascontiguousarray(cosT[:, last].astype(np.float32))
    sinT2 = np.ascontiguousarray(sinT[:, last].astype(np.float32))
    cosT = np.ascontiguousarray(cosT.astype(np.float32))
    sinT = np.ascontiguousarray(sinT.astype(np.float32))

    am = (inputs["attention_mask"] != 0)               # [B, S]
    tk = np.arange(c.S)
    m1 = np.zeros((c.B, c.SP, P, c.S), np.float32)
    for b in range(c.B):
        for t in range(c.SP):
            rows = tk[t * P:(t + 1) * P]
            m1[b, t] = ((rows[:, None] <= tk[None, :]) & am[b, rows][:, None])
    m1 = m1.astype(BF16)
    am2 = np.zeros((c.B, P, c.SP), np.float32)
    for b in range(c.B):
        am2[b] = am[b].reshape(c.SP, P).T
    am2 = am2.astype(BF16)

    # layernorm weights [5, P, KT]
    lnw = np.zeros((2 * c.L + 1, P, c.KT), np.float32)
    for l in range(c.L):
        lnw[2 * l] = inputs["ln1_w"][l].reshape(c.KT, P).T
        lnw[2 * l + 1] = inputs["ln2_w"][l].reshape(c.KT, P).T
    lnw[2 * c.L] = inputs["final_ln_w"].reshape(c.KT, P).T

    # cls head
    w1t = _wt3(inputs["w1"].astype(np.float32).T, P)       # [KT, P, CLS]
    b1c = np.ascontiguousarray(
        inputs["b1"].reshape(c.CT, P).T.astype(np.float32))
    gcol = np.ascontiguousarray(
        inputs["ln_g"].reshape(c.CT, P).T.astype(np.float32))
    bcol = np.ascontiguousarray(
        inputs["ln_b"].reshape(c.CT, P).T.astype(np.float32))
    w2t = np.ascontiguousarray(
        inputs["w2"].astype(np.float32).T.reshape(c.CT, P, c.NCLS).astype(BF16))
    b2c = np.ascontiguousarray(
        inputs["b2"].reshape(c.NCLS, 1).astype(np.float32))

    shared = dict(x0=x0, cosT=cosT, sinT=sinT, cosT2=cosT2, sinT2=sinT2,
                  m1=m1, am2=am2, lnw=lnw, w1t=w1t, b1c=b1c, gcol=gcol,
                  bcol=bcol, w2t=w2t, b2c=b2c)

    # per-layer dequantized weights (full) then shard
    per_layer = []
    for l in range(c.L):
        wqkv = dequant_np(inputs["qkv_packed"][l], inputs["qkv_absmax"][l],
                          3 * c.H, c.H)
        wo = dequant_np(inputs["o_packed"][l], inputs["o_absmax"][l],
                        c.H, c.H)
        wgu = dequant_np(inputs["gu_packed"][l], inputs["gu_absmax"][l],
                         2 * c.FF, c.H)
        wd = dequant_np(inputs["down_packed"][l], inputs["down_absmax"][l],
                        c.H, c.FF)
        per_layer.append((wqkv, wo, wgu, wd))

    in_maps = []
    for core in range(c.NC):
        m = dict(shared)
        m["x0r"] = np.ascontiguousarray(
            x0f[core * c.OR:(core + 1) * c.OR, :])
        for l in range(c.L):
            wqkv, wo, wgu, wd = per_layer[l]
            d0 = core * c.DR
            m[f"wq{l}"] = _wt3(wqkv[d0:d0 + c.DR, :].T, P)
            m[f"wk{l}"] = _wt3(wqkv[c.H + d0:c.H + d0 + c.DR, :].T, P)
            m[f"wv{l}"] = _wt3(wqkv[2 * c.H + d0:2 * c.H + d0 + c.DR, :].T, P)
            o0 = core * c.OR
            m[f"wo{l}"] = _wt3(wo[o0:o0 + c.OR, :].T, P)
            g0 = core * c.FPC
            m[f"wg{l}"] = _wt3(wgu[g0:g0 + c.FPC, :].T, P)
            m[f"wu{l}"] = _wt3(wgu[c.FF + g0:c.FF + g0 + c.FPC, :].T, P)
            m[f"wd{l}"] = _wt3(wd[o0:o0 + c.OR, :].T, P)
        in_maps.append(m)
    return in_maps


# ----------------------------------------------------------------------------
# device kernel
# ----------------------------------------------------------------------------

def build_nc(cfg: Cfg):
    import concourse.bass as bass
    import concourse.mybir as mybir
    import concourse.tile as tile
    from concourse import bacc

    c = cfg
    c.check()
    P = c.P
    f32 = mybir.dt.float32
    bf16 = mybir.dt.bfloat16
    AF = mybir.ActivationFunctionType
    OP = mybir.AluOpType

    nc = bacc.Bacc("TRN2", target_bir_lowering=False, debug=False,
                   enable_asserts=False, num_devices=c.NC)
    RG = [list(range(c.NC))]
    SHARED = "Shared" if c.NC > 4 else "Local"

    def din(name, shape, dt):
        return nc.dram_tensor(name, list(shape), dt, kind="ExternalInput").ap()

    x0 = din("x0", [c.H, c.T], bf16)
    x0r = din("x0r", [c.OR, c.T], f32)
    cosT = din("cosT", [c.HD, c.T], f32)
    sinT = din("sinT", [c.HD, c.T], f32)
    cosT2 = din("cosT2", [c.HD, c.B], f32)
    sinT2 = din("sinT2", [c.HD, c.B], f32)
    m1 = din("m1", [c.B, c.SP, P, c.S], bf16)
    am2 = din("am2", [c.B, P, c.SP], bf16)
    lnw_d = din("lnw", [2 * c.L + 1, P, c.KT], f32)
    w1t = din("w1t", [c.KT, P, c.CLS], bf16)
    b1c = din("b1c", [P, c.CT], f32)
    gcol = din("gcol", [P, c.CT], f32)
    bcol = din("bcol", [P, c.CT], f32)
    w2t = din("w2t", [c.CT, P, c.NCLS], bf16)
    b2c = din("b2c", [c.NCLS, 1], f32)
    wq = [din(f"wq{l}", [c.KT, P, c.DR], bf16) for l in range(c.L)]
    wk = [din(f"wk{l}", [c.KT, P, c.DR], bf16) for l in range(c.L)]
    wv = [din(f"wv{l}", [c.KT, P, c.DR], bf16) for l in range(c.L)]
    wo = [din(f"wo{l}", [c.KT, P, c.OR], bf16) for l in range(c.L)]
    wg = [din(f"wg{l}", [c.KT, P, c.FPC], bf16) for l in range(c.L)]
    wu = [din(f"wu{l}", [c.KT, P, c.FPC], bf16) for l in range(c.L)]
    wd = [din(f"wd{l}", [c.KTF, P, c.OR], bf16) for l in range(c.L)]
    out_d = nc.dram_tensor("logits_out", [c.NCLS, c.B], f32,
                           kind="ExternalOutput").ap()

    isqrt_hd = 1.0 / math.sqrt(c.HD)

    def lastcols(ap2d):
        """[P, T] AP -> [P, B] AP selecting the last token of each batch."""
        return ap2d.rearrange("p (b s) -> p b s", s=c.S)[:, :, c.S - 1]

    with tile.TileContext(nc) as tc, ExitStack() as ctx:
        const = ctx.enter_context(tc.tile_pool(name="const", bufs=1))
        persist = ctx.enter_context(tc.tile_pool(name="persist", bufs=1))
        wpool = ctx.enter_context(tc.tile_pool(name="wpool", bufs=3))
        xpool = ctx.enter_context(tc.tile_pool(name="xpool", bufs=3))
        spool = ctx.enter_context(tc.tile_pool(name="spool", bufs=2))
        ppool = ctx.enter_context(tc.tile_pool(name="ppool", bufs=3))
        rpool = ctx.enter_context(tc.tile_pool(name="rpool", bufs=1))
        psum = ctx.enter_context(tc.tile_pool(name="psum", bufs=8,
                                              space="PSUM"))
        dram = ctx.enter_context(tc.tile_pool(name="dram", bufs=1,
                                              space="DRAM"))

        # ---- constants in SBUF ----
        ones_c32 = const.tile([P, 1], f32, tag="ones_c32")
        nc.vector.memset(ones_c32[:], 1.0)
        ones_cbf = const.tile([P, 1], bf16, tag="ones_cbf")
        nc.vector.memset(ones_cbf[:], 1.0)
        ones_r32 = const.tile([1, P], f32, tag="ones_r32")
        nc.vector.memset(ones_r32[:], 1.0)
        eps_col = const.tile([P, 1], f32, tag="eps_col")
        nc.vector.memset(eps_col[:], EPS)
        cos_sb = const.tile([c.HD, c.T], f32, tag="cos_sb")
        nc.sync.dma_start(out=cos_sb[:], in_=cosT)
        sin_sb = const.tile([c.HD, c.T], f32, tag="sin_sb")
        nc.sync.dma_start(out=sin_sb[:], in_=sinT)
        cos2_sb = const.tile([c.HD, c.B], f32, tag="cos2_sb")
        nc.sync.dma_start(out=cos2_sb[:], in_=cosT2)
        sin2_sb = const.tile([c.HD, c.B], f32, tag="sin2_sb")
        nc.sync.dma_start(out=sin2_sb[:], in_=sinT2)
        mask_sb = const.tile([P, c.B * c.SP, c.S], bf16, tag="mask_sb")
        for b in range(c.B):
            for t in range(c.SP):
                nc.sync.dma_start(out=mask_sb[:, b * c.SP + t, :],
                                  in_=m1[b, t])
        am2_sb = const.tile([P, c.B, c.SP], bf16, tag="am2_sb")
        for b in range(c.B):
            nc.sync.dma_start(out=am2_sb[:, b, :], in_=am2[b])
        lnw_sb = const.tile([P, 2 * c.L + 1, c.KT], f32, tag="lnw_sb")
        for n in range(2 * c.L + 1):
            nc.sync.dma_start(out=lnw_sb[:, n, :], in_=lnw_d[n])
        b1_sb = const.tile([P, c.CT], f32, tag="b1_sb")
        nc.sync.dma_start(out=b1_sb[:], in_=b1c)
        g_sb_c = const.tile([P, c.CT], f32, tag="g_sb_c")
        nc.sync.dma_start(out=g_sb_c[:], in_=gcol)
        bcol_sb = const.tile([P, c.CT], f32, tag="bcol_sb")
        nc.sync.dma_start(out=bcol_sb[:], in_=bcol)
        b2_sb = const.tile([c.NCLS, 1], f32, tag="b2_sb")
        nc.sync.dma_start(out=b2_sb[:], in_=b2c)

        # ---- collective warm-up: absorb channel-establish cost under
        # the first compute phase (first real AG otherwise pays ~200us) ----
        wu_sb = const.tile([P, 512], f32, tag="wu_sb")
        nc.vector.memset(wu_sb[:], 0.0)
        wu_in = dram.tile([P, 512], f32, tag="wu_in", name="wu_in")
        wu_out = dram.tile([P * c.NC, 512], f32, addr_space=SHARED,
                           tag="wu_out", name="wu_out")
        nc.sync.dma_start(out=wu_in[:], in_=wu_sb[:])
        nc.gpsimd.collective_compute(
            "AllGather", OP.bypass, replica_groups=RG,
            ins=[wu_in[:]], outs=[wu_out[:]])

        # ---- persistent activation state ----
        xn = persist.tile([P, c.KT, c.T], bf16, tag="xn")       # normalized x (bf16)
        xrows = persist.tile([P, c.OT, c.T], f32, tag="xrows")     # this core's rows of x
        for ot in range(c.OT):
            nc.sync.dma_start(out=xrows[:, ot, :],
                              in_=x0r[ot * P:(ot + 1) * P, :])

        # ---------- helpers ----------
        def emit_norm(src_ap, lnidx, dst, ncols, chunks):
            """rmsnorm of src [H, ncols] (bf16) -> dst [P, KT, ncols] (bf16)."""
            ss = [psum.tile([1, cw], f32, tag="ps", name=f"ssps{lnidx}_{ci}")
                  for ci, (c0, cw) in enumerate(chunks)]
            for kt in range(c.KT):
                xf = xpool.tile([P, ncols], bf16, tag="xf", name="xf", bufs=2)
                nc.sync.dma_start(out=xf[:], in_=src_ap[kt * P:(kt + 1) * P, :])
                nc.vector.tensor_copy(dst[:, kt, :], xf[:])
                sq = xpool.tile([P, ncols], bf16, tag="sq", name="sq", bufs=2)
                nc.vector.tensor_mul(sq[:], xf[:], xf[:])
                for ci, (c0, cw) in enumerate(chunks):
                    nc.tensor.matmul(ss[ci][:], ones_cbf[:], sq[:, c0:c0 + cw],
                                     start=(kt == 0), stop=(kt == c.KT - 1))
            bc = spool.tile([P, ncols], f32, tag="bc", name="bc", bufs=1)
            for ci, (c0, cw) in enumerate(chunks):
                lt = spool.tile([1, cw], f32, tag="lt", name="lt")
                nc.scalar.activation(lt[:], ss[ci][:], AF.Ln,
                                     bias=eps_col[0:1, :], scale=1.0 / c.H)
                rt = spool.tile([1, cw], f32, tag="rt", name="rt")
                nc.scalar.activation(rt[:], lt[:], AF.Exp, scale=-0.5)
                bb = psum.tile([P, cw], f32, tag="ps", name="bbps")
                nc.tensor.matmul(bb[:], ones_r32[:], rt[:],
                                 start=True, stop=True)
                nc.scalar.copy(bc[:, c0:c0 + cw], bb[:])
            for kt in range(c.KT):
                nc.vector.scalar_tensor_tensor(
                    dst[:, kt, :], dst[:, kt, :],
                    lnw_sb[:, lnidx, kt:kt + 1], bc[:],
                    OP.mult, OP.mult)

        def emit_rstd_bcast(ss_aps, lnidx, ncols, chunks):
            """ss_aps: per-chunk [1, cw] APs of full-H sum-of-squares.
            Returns bc_sb [P, ncols] f32 with rsqrt(mean+eps) per token."""
            bc = spool.tile([P, ncols], f32, tag="bc", name="bc", bufs=1)
            for ci, (c0, cw) in enumerate(chunks):
                lt = spool.tile([1, cw], f32, tag="lt", name="lt")
                nc.scalar.activation(lt[:], ss_aps[ci], AF.Ln,
                                     bias=eps_col[0:1, :], scale=1.0 / c.H)
                rt = spool.tile([1, cw], f32, tag="rt", name="rt")
                nc.scalar.activation(rt[:], lt[:], AF.Exp, scale=-0.5)
                bb = psum.tile([P, cw], f32, tag="ps", name="bbps")
                nc.tensor.matmul(bb[:], ones_r32[:], rt[:],
                                 start=True, stop=True)
                nc.scalar.copy(bc[:, c0:c0 + cw], bb[:])
            return bc

        def emit_sumsq_ar(ncols, chunks, tag):
            """Partial sum-of-squares of this core's fp32 x rows, then a
            tiny AllReduce (issued before the x AllGather on the cc queue
            so the norm scale is ready when x tiles stream back)."""
            ssq = [psum.tile([1, cw], f32, tag="ps", name=f"ssA{tag}{ci}")
                   for ci, (c0, cw) in enumerate(chunks)]
            for ot in range(c.OT):
                sqr = xpool.tile([P, ncols], bf16, tag="sqr", name="sqr",
                                 bufs=2)
                nc.vector.tensor_mul(sqr[:], xrows[:, ot, :],
                                     xrows[:, ot, :])
                for ci, (c0, cw) in enumerate(chunks):
                    nc.tensor.matmul(ssq[ci][:], ones_cbf[:],
                                     sqr[:, c0:c0 + cw],
                                     start=(ot == 0), stop=(ot == c.OT - 1))
            srow = spool.tile([1, ncols], f32, tag="srow", name="srow")
            for ci, (c0, cw) in enumerate(chunks):
                nc.scalar.copy(srow[:, c0:c0 + cw], ssq[ci][:])
            ssb = dram.tile([1, ncols], f32, tag=f"ssb{tag}",
                            name=f"ssb{tag}")
            ssg = dram.tile([1, ncols], f32, addr_space=SHARED,
                            tag=f"ssg{tag}", name=f"ssg{tag}")
            nc.sync.dma_start(out=ssb[:], in_=srow[:])
            nc.gpsimd.collective_compute(
                "AllReduce", OP.add, replica_groups=RG,
                ins=[ssb[:]], outs=[ssg[:]])
            return ssg

        def emit_norm_post(ssg, src_ap, lnidx, dst, ncols, chunks):
            sst = spool.tile([1, ncols], f32, tag="sst", name="sst")
            nc.sync.dma_start(out=sst[:], in_=ssg[:])
            bc = emit_rstd_bcast(
                [sst[:, c0:c0 + cw] for (c0, cw) in chunks],
                lnidx, ncols, chunks)
            for kt in range(c.KT):
                xf = xpool.tile([P, ncols], bf16, tag="xf", name="xfa",
                                bufs=2)
                nc.sync.dma_start(out=xf[:],
                                  in_=src_ap[kt * P:(kt + 1) * P, :])
                nc.vector.scalar_tensor_tensor(
                    dst[:, kt, :], xf[:], lnw_sb[:, lnidx, kt:kt + 1],
                    bc[:], OP.mult, OP.mult)

        def emit_norm_slim(src_ap, lnidx, dst3):
            """rmsnorm of a [H, B] tensor: one DMA + local sumsq."""
            xs = spool.tile([P, c.KT, c.B], bf16, tag="xs_slim",
                            name="xs_slim")
            nc.sync.dma_start(
                out=xs[:],
                in_=src_ap.rearrange("(kt p) b -> p kt b", p=P))
            sq = spool.tile([P, c.KT, c.B], f32, tag="sq_slim",
                            name="sq_slim")
            nc.vector.tensor_mul(sq[:], xs[:], xs[:])
            sp_ = psum.tile([1, c.KT * c.B], f32, tag="ps", name="spslim")
            nc.tensor.matmul(sp_[:], ones_c32[:],
                             sq[:].rearrange("p kt b -> p (kt b)"),
                             start=True, stop=True)
            ss2 = spool.tile([1, c.B], f32, tag="ss2", name="ss2")
            nc.vector.tensor_reduce(
                ss2[:], sp_[:].rearrange("o (kt b) -> o b kt", b=c.B),
                mybir.AxisListType.X, OP.add)
            bc = emit_rstd_bcast([ss2[:]], lnidx, c.B, [(0, c.B)])
            tmp = spool.tile([P, c.KT, c.B], f32, tag="tmp_slim",
                             name="tmp_slim")
            nc.vector.tensor_tensor(
                tmp[:], xs[:],
                lnw_sb[:, lnidx, :].unsqueeze(2).broadcast_to(
                    (P, c.KT, c.B)), OP.mult)
            nc.vector.tensor_tensor(
                dst3[:], tmp[:],
                bc[:].unsqueeze(1).broadcast_to((P, c.KT, c.B)), OP.mult)

        def kouter_pass(KK, wsrc, wcols, groups, rhs_fn, rhs_load=None,
                        name="kp"):
            """Generic contraction pass: loop k tiles (batched weight DMA),
            stream weights, accumulate len(groups) psum tiles.
            groups: list of (lhs_c0, lhs_cw, out_n, rhs_key)."""
            ps = [psum.tile([cw, n], f32, tag="ps", name=f"{name}{gi}")
                  for gi, (c0, cw, n, rk) in enumerate(groups)]
            G = max(1, min(4, 2048 // wcols))
            for k0 in range(0, KK, G):
                g_n = min(G, KK - k0)
                wt = wpool.tile([P, G, 2048 // G if wcols > 2048 // G else wcols],
                                bf16, tag="wt", name=f"{name}w")
                nc.sync.dma_start(
                    out=wt[:, 0:g_n, 0:wcols],
                    in_=wsrc(k0, g_n).rearrange("g p m -> p g m"))
                for g in range(g_n):
                    kt = k0 + g
                    rl = rhs_load(kt) if rhs_load is not None else None
                    for gi, (c0, cw, n, rk) in enumerate(groups):
                        nc.tensor.matmul(ps[gi][:], wt[:, g, c0:c0 + cw],
                                         rhs_fn(kt, rk, rl),
                                         start=(kt == 0), stop=(kt == KK - 1))
            return ps

        def emit_rope(src_ps, qr_dst, cos_ap, sin_ap, ncols):
            """rope: qr_dst = src*cos + swap_half(src)*sin_signed."""
            h2 = c.HD // 2
            qs = rpool.tile([c.HD, ncols], f32, tag="qs", name="qs")
            nc.vector.tensor_copy(qs[:], src_ps[:])
            rot = rpool.tile([c.HD, ncols], f32, tag="rot", name="rot")
            nc.sync.dma_start(out=rot[0:h2, :], in_=qs[h2:c.HD, :])
            nc.sync.dma_start(out=rot[h2:c.HD, :], in_=qs[0:h2, :])
            nc.vector.tensor_mul(qs[:], qs[:], cos_ap)
            nc.vector.tensor_mul(rot[:], rot[:], sin_ap)
            nc.vector.tensor_add(qr_dst, qs[:], rot[:])

        # ================= transformer layers =================
        x_src = x0
        ln1_ssg = None
        for l in range(c.L):
            slim = (l == c.L - 1)
            ncol2 = c.B if slim else c.T
            full_chunks = [(b * c.S, c.S) for b in range(c.B)]

            # ---- ln1 + qkv ----
            if ln1_ssg is None:
                emit_norm(x_src, 2 * l, xn, c.T, full_chunks)
            else:
                emit_norm_post(ln1_ssg, x_src, 2 * l, xn, c.T, full_chunks)

            q_rot = persist.tile([c.HD, c.HPC, ncol2], bf16, tag="qrot",
                                 name=f"qrot{l}")
            k_rot = persist.tile([c.HD, c.HPC, c.T], bf16, tag="krot",
                                 name=f"krot{l}")
            v_sb = persist.tile([P, c.TP_, c.DR], bf16, tag="vsb",
                                name=f"vsb{l}")

            # q pass
            if slim:
                qg = [(h * c.HD, c.HD, c.B, 0) for h in range(c.HPC)]
                qrhs = lambda kt, rk, rl: lastcols(xn[:, kt, :])
            else:
                qg = [(h * c.HD, c.HD, c.S, b)
                      for h in range(c.HPC) for b in range(c.B)]
                qrhs = lambda kt, rk, rl: xn[:, kt, rk * c.S:(rk + 1) * c.S]
            qps = kouter_pass(c.KT, lambda k0, n: wq[l][k0:k0 + n], c.DR, qg, qrhs,
                              name="qp")
            for gi, (c0, cw, n, rk) in enumerate(qg):
                h = c0 // c.HD
                if slim:
                    emit_rope(qps[gi], q_rot[:, h, :], cos2_sb[:], sin2_sb[:],
                              c.B)
                else:
                    emit_rope(qps[gi], q_rot[:, h, rk * c.S:(rk + 1) * c.S],
                              cos_sb[:, rk * c.S:(rk + 1) * c.S],
                              sin_sb[:, rk * c.S:(rk + 1) * c.S], c.S)

            # k pass (always full)
            kg = [(h * c.HD, c.HD, c.S, b)
                  for h in range(c.HPC) for b in range(c.B)]
            krhs = lambda kt, rk, rl: xn[:, kt, rk * c.S:(rk + 1) * c.S]
            kps = kouter_pass(c.KT, lambda k0, n: wk[l][k0:k0 + n], c.DR, kg, krhs,
                              name="kp")
            for gi, (c0, cw, n, rk) in enumerate(kg):
                h = c0 // c.HD
                emit_rope(kps[gi], k_rot[:, h, rk * c.S:(rk + 1) * c.S],
                          cos_sb[:, rk * c.S:(rk + 1) * c.S],
                          sin_sb[:, rk * c.S:(rk + 1) * c.S], c.S)

            # v pass (token-major): psum groups per token tile
            vps = [psum.tile([P, c.DR], f32, tag="ps", name=f"vp{tt}")
                   for tt in range(c.TP_)]
            for kt in range(c.KT):
                wt = wpool.tile([P, c.DR], bf16, tag="wt", name="vw")
                nc.sync.dma_start(out=wt[:], in_=wv[l][kt])
                for tt in range(c.TP_):
                    nc.tensor.matmul(vps[tt][:],
                                     xn[:, kt, tt * P:(tt + 1) * P], wt[:],
                                     start=(kt == 0), stop=(kt == c.KT - 1))
            for tt in range(c.TP_):
                nc.scalar.copy(v_sb[:, tt, :], vps[tt][:])

            # ---- attention ----
            if slim:
                ctxb = dram.tile([c.DR, ncol2], bf16, tag=f"ctxb{l}",
                                 name=f"ctxb{l}")
                ctxg = dram.tile([c.H, ncol2], bf16, addr_space=SHARED,
                                 tag=f"ctxg{l}", name=f"ctxg{l}")
                ctxbs, ctxgs = [ctxb], [ctxg]
            else:
                ctxbs = [dram.tile([c.DR, c.S], bf16, tag=f"ctxb{l}_{b}",
                                   name=f"ctxb{l}_{b}") for b in range(c.B)]
                ctxgs = [dram.tile([c.H, c.S], bf16, addr_space=SHARED,
                                   tag=f"ctxg{l}_{b}", name=f"ctxg{l}_{b}")
                         for b in range(c.B)]
            for b in range(c.B):
                for h in range(c.HPC):
                    if not slim:
                        den = psum.tile([1, c.S], f32, tag="ps", name="den")
                        cps = psum.tile([c.HD, c.S], f32, tag="ps", name="cps")
                        for t in range(c.SP):
                            sps = psum.tile([P, c.S], f32, tag="ps",
                                            name="sps")
                            nc.tensor.matmul(
                                sps[:],
                                k_rot[:, h, b * c.S + t * P:
                                      b * c.S + (t + 1) * P],
                                q_rot[:, h, b * c.S:(b + 1) * c.S],
                                start=True, stop=True)
                            pt = ppool.tile([P, c.S], bf16, tag="pt",
                                            name="pt")
                            nc.scalar.activation(pt[:], sps[:], AF.Exp,
                                                 scale=isqrt_hd)
                            nc.vector.tensor_mul(
                                pt[:], pt[:], mask_sb[:, b * c.SP + t, :])
                            nc.tensor.matmul(den[:], ones_cbf[:], pt[:],
                                             start=(t == 0),
                                             stop=(t == c.SP - 1))
                            nc.tensor.matmul(
                                cps[:],
                                v_sb[:, b * c.SP + t,
                                     h * c.HD:(h + 1) * c.HD],
                                pt[:],
                                start=(t == 0), stop=(t == c.SP - 1))
                        dr = spool.tile([1, c.S], f32, tag="dr", name="dr")
                        nc.vector.reciprocal(dr[:], den[:])
                        bb = psum.tile([c.HD, c.S], f32, tag="ps", name="bb")
                        nc.tensor.matmul(bb[:], ones_r32[:, 0:c.HD], dr[:],
                                         start=True, stop=True)
                        bsb = spool.tile([c.HD, c.S], f32, tag="bsb",
                                         name="bsb", bufs=2)
                        nc.vector.tensor_copy(bsb[:], bb[:])
                        csb = spool.tile([c.HD, c.S], bf16, tag="csb",
                                         name="csb", bufs=2)
                        nc.vector.tensor_mul(csb[:], cps[:], bsb[:])
                        nc.sync.dma_start(
                            out=ctxbs[b][h * c.HD:(h + 1) * c.HD, :],
                            in_=csb[:])
                    else:
                        sps = psum.tile([P, c.SP], f32, tag="ps", name="sps2")
                        for t in range(c.SP):
                            nc.tensor.matmul(
                                sps[:, t:t + 1],
                                k_rot[:, h, b * c.S + t * P:
                                      b * c.S + (t + 1) * P],
                                q_rot[:, h, b:b + 1],
                                start=True, stop=True)
                        pt = ppool.tile([P, c.SP], bf16, tag="pt2",
                                        name="pt2")
                        nc.scalar.activation(pt[:], sps[:], AF.Exp,
                                             scale=isqrt_hd)
                        nc.vector.tensor_mul(pt[:], pt[:], am2_sb[:, b, :])
                        dps = psum.tile([1, c.SP], f32, tag="ps", name="dps")
                        nc.tensor.matmul(dps[:], ones_cbf[:], pt[:],
                                         start=True, stop=True)
                        d1 = spool.tile([1, 1], f32, tag="d1", name="d1")
                        nc.vector.tensor_reduce(d1[:], dps[:],
                                                mybir.AxisListType.X, OP.add)
                        r1 = spool.tile([1, 1], f32, tag="r1", name="r1")
                        nc.vector.reciprocal(r1[:], d1[:])
                        cps = psum.tile([c.HD, 1], f32, tag="ps", name="cps2")
                        for t in range(c.SP):
                            nc.tensor.matmul(
                                cps[:],
                                v_sb[:, b * c.SP + t,
                                     h * c.HD:(h + 1) * c.HD],
                                pt[:, t:t + 1],
                                start=(t == 0), stop=(t == c.SP - 1))
                        bb = psum.tile([c.HD, 1], f32, tag="ps", name="bb2")
                        nc.tensor.matmul(bb[:], ones_r32[:, 0:c.HD], r1[:],
                                         start=True, stop=True)
                        bsb = spool.tile([c.HD, 1], f32, tag="bsb2",
                                         name="bsb2")
                        nc.vector.tensor_copy(bsb[:], bb[:])
                        csb = spool.tile([c.HD, 1], bf16, tag="csb2",
                                         name="csb2")
                        nc.vector.tensor_mul(csb[:], cps[:], bsb[:])
                        nc.sync.dma_start(
                            out=ctxbs[0][h * c.HD:(h + 1) * c.HD, b:b + 1],
                            in_=csb[:])
                if not slim:
                    nc.gpsimd.collective_compute(
                        "AllGather", OP.bypass, replica_groups=RG,
                        ins=[ctxbs[b][:]], outs=[ctxgs[b][:]])
            if slim:
                nc.gpsimd.collective_compute(
                    "AllGather", OP.bypass, replica_groups=RG,
                    ins=[ctxbs[0][:]], outs=[ctxgs[0][:]])

            # ---- o projection (+ residual into xrows) ----
            if slim:
                og = [(ot * P, P, c.B, 0) for ot in range(c.OT)]
                orhs = lambda kt, rk, rl: rl[:]

                def oload(kt):
                    t = xpool.tile([P, c.B], bf16, tag="orhs",
                                   name="orhs", bufs=3)
                    nc.scalar.dma_start(
                        out=t[:], in_=ctxgs[0][kt * P:(kt + 1) * P, :])
                    return t
                ops_ = kouter_pass(c.KT, lambda k0, n: wo[l][k0:k0 + n],
                                   c.OR, og, orhs, rhs_load=oload,
                                   name="op")
            else:
                # one pass per batch so batch-0 matmuls overlap batch-1's
                # ctx AllGather (weights streamed twice; 2.4MB extra)
                ops_, og = [], []
                for b in range(c.B):
                    og_b = [(ot * P, P, c.S, b) for ot in range(c.OT)]
                    orhs = lambda kt, rk, rl: rl[:]

                    def oload(kt, _b=b):
                        t = xpool.tile([P, c.S], bf16, tag="orhs",
                                       name="orhs", bufs=3)
                        nc.scalar.dma_start(
                            out=t[:],
                            in_=ctxgs[_b][kt * P:(kt + 1) * P, :])
                        return t
                    ops_ += kouter_pass(c.KT,
                                        lambda k0, n: wo[l][k0:k0 + n],
                                        c.OR, og_b, orhs, rhs_load=oload,
                                        name=f"op{b}")
                    og += og_b
            xbo = dram.tile([c.OR, ncol2], bf16, tag=f"xbo{l}", name=f"xbo{l}")
            xgo = dram.tile([c.H, ncol2], bf16, addr_space=SHARED,
                            tag=f"xgo{l}", name=f"xgo{l}")
            for gi, (c0, cw, n, rk) in enumerate(og):
                ot = c0 // P
                if slim:
                    xsl = lastcols(xrows[:, ot, :])
                else:
                    xsl = xrows[:, ot, rk * c.S:(rk + 1) * c.S]
                nc.vector.tensor_add(xsl, xsl, ops_[gi][:])
                st = xpool.tile([P, n], bf16, tag="xst", name="xst", bufs=2)
                nc.scalar.copy(st[:], xsl)
                if slim:
                    nc.sync.dma_start(out=xbo[ot * P:(ot + 1) * P, :],
                                      in_=st[:])
                else:
                    nc.sync.dma_start(
                        out=xbo[ot * P:(ot + 1) * P,
                                rk * c.S:(rk + 1) * c.S],
                        in_=st[:])
            if not slim:
                ln2_ssg = emit_sumsq_ar(c.T, full_chunks, tag=f"o{l}")
            nc.gpsimd.collective_compute(
                "AllGather", OP.bypass, replica_groups=RG,
                ins=[xbo[:]], outs=[xgo[:]])

            # ---- ln2 + MLP ----
            if slim:
                xn2 = persist.tile([P, c.KT, c.B], bf16, tag="xn2",
                                   name="xn2")
                emit_norm_slim(xgo[:], 2 * l + 1, xn2)
                mlp_src = xn2
                mchunks = [(0, c.B)]
            else:
                emit_norm_post(ln2_ssg, xgo[:], 2 * l + 1, xn, c.T,
                               full_chunks)
                mlp_src = xn
                mchunks = full_chunks

            gact = persist.tile([P, c.FT, ncol2], bf16, tag="gact",
                                name=f"gact{l}")
            max_ot = max(1, 8 // len(mchunks))
            intb = dram.tile([c.FPC, ncol2], bf16, tag=f"intb{l}", name=f"intb{l}")
            intg = dram.tile([c.FF, ncol2], bf16, addr_space=SHARED,
                             tag=f"intg{l}", name=f"intg{l}")
            for phase, wsrc3 in (("g", wg[l]), ("u", wu[l])):
                for ot0 in range(0, c.FT, max_ot):
                    ots = range(ot0, min(ot0 + max_ot, c.FT))
                    gg = [(ot * P - ot0 * P, P, cw, (ot, ci))
                          for ot in ots for ci, (cc0, cw) in
                          enumerate(mchunks)]
                    wcols = len(ots) * P

                    def gsrc(k0, n, _w=wsrc3, _c0=ot0 * P, _cw=wcols):
                        return _w[k0:k0 + n, :, _c0:_c0 + _cw]
                    grhs = (lambda kt, rk, rl:
                            mlp_src[:, kt,
                                    mchunks[rk[1]][0]:
                                    mchunks[rk[1]][0] + mchunks[rk[1]][1]])
                    gps = kouter_pass(c.KT, gsrc, wcols, gg, grhs,
                                      name=f"{phase}{ot0}")
                    for gi, (c0, cw, n, rk) in enumerate(gg):
                        ot, ci = rk
                        cc0 = mchunks[ci][0]
                        if phase == "g":
                            sgt = xpool.tile([P, n], bf16, tag="sgt",
                                             name="sgt", bufs=2)
                            nc.scalar.activation(sgt[:], gps[gi][:],
                                                 AF.Sigmoid)
                            nc.vector.tensor_mul(
                                gact[:, ot, cc0:cc0 + n], gps[gi][:],
                                sgt[:])
                        else:
                            it = xpool.tile([P, n], bf16, tag="it",
                                            name="it", bufs=2)
                            nc.vector.tensor_mul(
                                it[:], gps[gi][:], gact[:, ot, cc0:cc0 + n])
                            nc.sync.dma_start(
                                out=intb[ot * P:(ot + 1) * P,
                                         cc0:cc0 + n],
                                in_=it[:])
            nc.gpsimd.collective_compute(
                "AllGather", OP.bypass, replica_groups=RG,
                ins=[intb[:]], outs=[intg[:]])

            # ---- down projection (+ residual) ----
            if slim:
                dg = [(ot * P, P, c.B, 0) for ot in range(c.OT)]
                drhs = lambda kt, rk, rl: rl[:]

            else:
                dg = [(ot * P, P, c.S, b)
                      for ot in range(c.OT) for b in range(c.B)]
                drhs = lambda kt, rk, rl: rl[:, rk * c.S:(rk + 1) * c.S]

            def dload(kt):
                t = xpool.tile([P, ncol2], bf16, tag="drhs", name="drhs",
                               bufs=3)
                nc.scalar.dma_start(out=t[:],
                                    in_=intg[kt * P:(kt + 1) * P, :])
                return t
            dps_ = kouter_pass(c.KTF, lambda k0, n: wd[l][k0:k0 + n], c.OR, dg, drhs,
                               rhs_load=dload, name="dp")
            xbd = dram.tile([c.OR, ncol2], bf16, tag=f"xbd{l}", name=f"xbd{l}")
            xgd = dram.tile([c.H, ncol2], bf16, addr_space=SHARED,
                            tag=f"xgd{l}", name=f"xgd{l}")
            for gi, (c0, cw, n, rk) in enumerate(dg):
                ot = c0 // P
                if slim:
                    xsl = lastcols(xrows[:, ot, :])
                else:
                    xsl = xrows[:, ot, rk * c.S:(rk + 1) * c.S]
                nc.vector.tensor_add(xsl, xsl, dps_[gi][:])
                st = xpool.tile([P, n], bf16, tag="xst", name="xst2", bufs=2)
                nc.scalar.copy(st[:], xsl)
                if slim:
                    nc.sync.dma_start(out=xbd[ot * P:(ot + 1) * P, :],
                                      in_=st[:])
                else:
                    nc.sync.dma_start(
                        out=xbd[ot * P:(ot + 1) * P,
                                rk * c.S:(rk + 1) * c.S],
                        in_=st[:])
            if not slim:
                ln1_ssg = emit_sumsq_ar(c.T, full_chunks, tag=f"d{l}")
            nc.gpsimd.collective_compute(
                "AllGather", OP.bypass, replica_groups=RG,
                ins=[xbd[:]], outs=[xgd[:]])
            x_src = xgd[:]

        # ================= final norm + cls head =================
        xnf = persist.tile([P, c.KT, c.B], bf16, tag="xn2", name="xnf")
        emit_norm_slim(x_src, 2 * c.L, xnf)

        hps = [psum.tile([P, c.B], f32, tag="ps", name=f"hps{ot}")
               for ot in range(c.CT)]
        for kt in range(c.KT):
            wt = wpool.tile([P, c.CLS], bf16, tag="w1w", name="w1w",
                            bufs=2)
            nc.sync.dma_start(out=wt[:], in_=w1t[kt])
            for ot in range(c.CT):
                nc.tensor.matmul(hps[ot][:], wt[:, ot * P:(ot + 1) * P],
                                 xnf[:, kt, :],
                                 start=(kt == 0), stop=(kt == c.KT - 1))
        h_sb = persist.tile([P, c.CT, c.B], bf16, tag="h_sb", name="h_sb")
        mn = psum.tile([1, c.B], f32, tag="ps", name="mn")
        ssq = psum.tile([1, c.B], f32, tag="ps", name="ssq")
        for ot in range(c.CT):
            nc.scalar.activation(h_sb[:, ot, :], hps[ot][:], AF.Relu,
                                 bias=b1_sb[:, ot:ot + 1])
            hq = spool.tile([P, c.B], f32, tag="hq", name="hq")
            nc.vector.tensor_mul(hq[:], h_sb[:, ot, :], h_sb[:, ot, :])
            nc.tensor.matmul(mn[:], ones_cbf[:], h_sb[:, ot, :],
                             start=(ot == 0), stop=(ot == c.CT - 1))
            nc.tensor.matmul(ssq[:], ones_c32[:], hq[:],
                             start=(ot == 0), stop=(ot == c.CT - 1))
        m_sb = spool.tile([1, c.B], f32, tag="m_sb", name="m_sb")
        nc.vector.tensor_scalar_mul(m_sb[:], mn[:], 1.0 / c.CLS)
        s_sb = spool.tile([1, c.B], f32, tag="s_sb", name="s_sb")
        nc.vector.tensor_scalar_mul(s_sb[:], ssq[:], 1.0 / c.CLS)
        msq = spool.tile([1, c.B], f32, tag="msq", name="msq")
        nc.vector.tensor_mul(msq[:], m_sb[:], m_sb[:])
        var = spool.tile([1, c.B], f32, tag="var", name="var")
        nc.vector.tensor_sub(var[:], s_sb[:], msq[:])
        lv = spool.tile([1, c.B], f32, tag="lv", name="lv")
        nc.scalar.activation(lv[:], var[:], AF.Ln, bias=eps_col[0:1, :])
        rstd = spool.tile([1, c.B], f32, tag="rstd", name="rstd")
        nc.scalar.activation(rstd[:], lv[:], AF.Exp, scale=-0.5)
        bmp = psum.tile([P, c.B], f32, tag="ps", name="bmp")
        nc.tensor.matmul(bmp[:], ones_r32[:], m_sb[:], start=True, stop=True)
        bm_sb = spool.tile([P, c.B], f32, tag="bm", name="bm")
        nc.vector.tensor_copy(bm_sb[:], bmp[:])
        brp = psum.tile([P, c.B], f32, tag="ps", name="brp")
        nc.tensor.matmul(brp[:], ones_r32[:], rstd[:], start=True, stop=True)
        br_sb = spool.tile([P, c.B], f32, tag="br", name="br")
        nc.vector.tensor_copy(br_sb[:], brp[:])

        lg = psum.tile([c.NCLS, c.B], f32, tag="ps", name="lg")
        for ot in range(c.CT):
            t1 = spool.tile([P, c.B], f32, tag="ct1", name="ct1")
            nc.vector.tensor_sub(t1[:], h_sb[:, ot, :], bm_sb[:])
            t2 = spool.tile([P, c.B], f32, tag="ct2", name="ct2")
            nc.vector.tensor_mul(t2[:], t1[:], br_sb[:])
            hn = spool.tile([P, c.B], bf16, tag="hn", name="hn")
            nc.vector.tensor_scalar(hn[:], t2[:], g_sb_c[:, ot:ot + 1],
                                    bcol_sb[:, ot:ot + 1], OP.mult, OP.add)
            w2w = wpool.tile([P, c.NCLS], bf16, tag="w2w", name="w2w")
            nc.sync.dma_start(out=w2w[:], in_=w2t[ot])
            nc.tensor.matmul(lg[:], w2w[:], hn[:],
                             start=(ot == 0), stop=(ot == c.CT - 1))
        lg_sb = spool.tile([c.NCLS, c.B], f32, tag="lg_sb", name="lg_sb")
        nc.vector.tensor_scalar(lg_sb[:], lg[:], b2_sb[:], None, OP.add)
        nc.sync.dma_start(out=out_d, in_=lg_sb[:])

    nc.compile()
    return nc


# ----------------------------------------------------------------------------
# entry point
# ----------------------------------------------------------------------------

_CACHE = {}


def _get_nc(cfg):
    if cfg not in _CACHE:
        _CACHE[cfg] = build_nc(cfg)
    return _CACHE[cfg]


def run(cfg, inputs, trace=False, **kw):
    from concourse.bass_utils import run_bass_kernel_spmd
    in_maps = host_prep(cfg, inputs)
    nc = _get_nc(cfg)
    res = run_bass_kernel_spmd(nc, in_maps, core_ids=list(range(cfg.NC)),
                               trace=trace, **kw)
    out = np.asarray(res.results[0]["logits_out"])  # [NCLS, B]
    return np.ascontiguousarray(out.T.astype(np.float32)), res


def kernel(**inputs):
    inputs = {k: np.asarray(v) for k, v in inputs.items()}
    out, _ = run(FULL_CFG, inputs)
    return out



# revision 11
# speedup vs baseline: 1.2319x; 1.2319x over previous
"""Trainium2 Bass kernel for nn_Net_89361089561102 (2-layer dense transformer,
NF4-quantized weights, cls head). Tensor-parallel over 8 NeuronCores.

Strategy (v2, Megatron-style):
 - Host: unpack NF4 -> bf16 weights in flat [P, cols] layouts for big
   contiguous DMA streams; embedding gather + layer-0 ln1 precomputed.
 - qkv/gate_up sharded by output dim; o/down sharded by INPUT dim, so the
   per-core attention ctx / MLP intermediate stay local and the only layer
   collectives are fp16 AllReduces of the (residual-carrying) output, one
   per batch, overlapped with the other batch's compute.
 - Residual x is carried inside the AllReduce: every core contributes
   x_prev/8 on top of its partial projection.
 - Layer 2 (last layer): q/o/MLP evaluated only at the last token of each
   batch; k/v full. Tiny single AllReduces.
 - rope rotate-half via a PE permutation matmul (no DMAs).
"""

import math
from contextlib import ExitStack
from dataclasses import dataclass

import numpy as np
import ml_dtypes

BF16 = ml_dtypes.bfloat16
FP16 = np.float16
EPS = 1e-5
BLK = 64
NF4 = np.array([
    -1.0, -0.6961928009986877, -0.5250730514526367, -0.39491748809814453,
    -0.28444138169288635, -0.18477343022823334, -0.09105003625154495, 0.0,
    0.07958029955625534, 0.16093020141124725, 0.24611230194568634,
    0.33791524171829224, 0.44070982933044434, 0.5626170039176941,
    0.7229568362236023, 1.0], dtype=np.float32)


@dataclass(frozen=True)
class Cfg:
    H: int
    NH: int
    HD: int
    FF: int
    B: int
    S: int
    L: int
    NC: int
    CLS: int = 768
    NCLS: int = 2
    P: int = 128

    @property
    def T(self):
        return self.B * self.S

    @property
    def KT(self):
        return self.H // self.P

    @property
    def HPC(self):  # heads per core
        return self.NH // self.NC

    @property
    def DR(self):  # q/k/v rows per core
        return self.HPC * self.HD

    @property
    def FPC(self):  # ff rows per core
        return self.FF // self.NC

    @property
    def FT(self):
        return self.FPC // self.P

    @property
    def SP(self):  # seq tiles per batch
        return self.S // self.P

    @property
    def CT(self):
        return self.CLS // self.P

    def check(self):
        assert self.H % self.P == 0 and self.FF % self.P == 0
        assert self.S % self.P == 0 and self.S <= 512
        assert self.NH % self.NC == 0 and self.H % self.NC == 0
        assert self.FF % self.NC == 0 and self.FPC % self.P == 0
        assert self.HD <= self.P and self.HD % 2 == 0
        assert self.HPC * self.B <= 8   # q/k psum groups
        assert self.SP * self.B <= 8    # v psum groups
        assert self.FT <= 8             # gu psum groups
        assert self.CT <= 8             # cls psum groups
        assert self.CLS % self.P == 0


FULL_CFG = Cfg(H=3072, NH=32, HD=96, FF=8192, B=2, S=512, L=2, NC=8)


# ----------------------------------------------------------------------------
# host-side prep
# ----------------------------------------------------------------------------

def dequant_np(packed, absmax, out_f, in_f):
    shifts = (np.arange(8, dtype=np.int32) * 4)
    codes = ((packed[:, None] >> shifts) & 0xF).reshape(-1)
    w = (NF4[codes].reshape(-1, BLK) * absmax[:, None].astype(np.float32))
    return w.reshape(out_f, in_f)


def _flat_kpm(wt, P):
    """[K, M] -> flat [P, (K//P)*M] bf16: [p, kt*M+m] = wt[kt*P+p, m]."""
    K, M = wt.shape
    a = wt.reshape(K // P, P, M).transpose(1, 0, 2).reshape(P, (K // P) * M)
    return np.ascontiguousarray(a.astype(BF16))


def host_prep(cfg: Cfg, inputs):
    """Full inputs -> list of per-core input maps."""
    c = cfg
    P = c.P
    x = inputs["embed"][inputs["input_ids"]]          # [B, S, H] fp32
    x0f = np.ascontiguousarray(
        x.reshape(c.T, c.H).T.astype(np.float32))      # [H, T]

    # layer-0 ln1 on host
    rstd0 = 1.0 / np.sqrt((x0f * x0f).mean(0) + EPS)   # [T]
    xn0 = x0f * rstd0[None, :] * inputs["ln1_w"][0].astype(np.float32)[:, None]
    xn0h = np.ascontiguousarray(
        xn0.reshape(c.KT, P, c.T).transpose(1, 0, 2)
        .reshape(P, c.KT * c.T).astype(BF16))
    xaddh = np.ascontiguousarray(
        (x0f / c.NC).reshape(c.KT, P, c.T).transpose(1, 0, 2)
        .reshape(P, c.KT * c.T).astype(FP16))

    # rope tables (sign folded into sin)
    inv = 1.0 / (10000.0 ** (np.arange(0, c.HD, 2, dtype=np.float32) / c.HD))
    f = np.outer(np.arange(c.S, dtype=np.float32), inv)
    emb = np.concatenate([f, f], -1)                   # [S, HD]
    sgn = np.concatenate([-np.ones(c.HD // 2, np.float32),
                          np.ones(c.HD // 2, np.float32)])
    cosT = np.ascontiguousarray(
        np.tile(np.cos(emb).T, (1, c.B)).astype(np.float32))     # [HD, T]
    sinT = np.ascontiguousarray(
        np.tile(np.sin(emb).T * sgn[:, None], (1, c.B)).astype(np.float32))
    last = np.array([b * c.S + c.S - 1 for b in range(c.B)])
    cosT2 = np.ascontiguousarray(cosT[:, last])
    sinT2 = np.ascontiguousarray(sinT[:, last])

    # rope rotate-half permutation: rot[m] = q[m+h2] (m<h2) / q[m-h2]
    h2 = c.HD // 2
    perm = np.zeros((c.HD, c.HD), np.float32)          # [k, m] lhsT
    for m in range(c.HD):
        perm[(m + h2) % c.HD, m] = 1.0
    permh = np.ascontiguousarray(perm.astype(BF16))

    # masks: scores are [keys, queries]
    am = (inputs["attention_mask"] != 0)               # [B, S]
    tk = np.arange(c.S)
    m1 = np.zeros((P, c.B * c.SP * c.S), np.float32)
    for b in range(c.B):
        for t in range(c.SP):
            keys = tk[t * P:(t + 1) * P]
            blk = ((keys[:, None] <= tk[None, :]) & am[b, keys][:, None])
            m1[:, (b * c.SP + t) * c.S:(b * c.SP + t + 1) * c.S] = blk
    m1h = np.ascontiguousarray(m1.astype(BF16))
    am2 = np.zeros((P, c.B * c.SP), np.float32)
    for b in range(c.B):
        am2[:, b * c.SP:(b + 1) * c.SP] = am[b].reshape(c.SP, P).T
    am2h = np.ascontiguousarray(am2.astype(BF16))

    # layernorm weights [P, (2L+1)*KT]
    nln = 2 * c.L + 1
    lnw = np.zeros((P, nln * c.KT), np.float32)
    for l in range(c.L):
        lnw[:, (2 * l) * c.KT:(2 * l + 1) * c.KT] = \
            inputs["ln1_w"][l].reshape(c.KT, P).T
        lnw[:, (2 * l + 1) * c.KT:(2 * l + 2) * c.KT] = \
            inputs["ln2_w"][l].reshape(c.KT, P).T
    lnw[:, 2 * c.L * c.KT:] = inputs["final_ln_w"].reshape(c.KT, P).T
    lnwh = np.ascontiguousarray(lnw)

    # cls head
    w1h = _flat_kpm(inputs["w1"].astype(np.float32).T, P)   # [P, KT*CLS]
    w2h = _flat_kpm(inputs["w2"].astype(np.float32).T, P)   # [P, CT*NCLS]
    b1c = np.ascontiguousarray(
        inputs["b1"].reshape(c.CT, P).T.astype(np.float32))
    gcol = np.ascontiguousarray(
        inputs["ln_g"].reshape(c.CT, P).T.astype(np.float32))
    bcol = np.ascontiguousarray(
        inputs["ln_b"].reshape(c.CT, P).T.astype(np.float32))
    b2c = np.ascontiguousarray(
        inputs["b2"].reshape(c.NCLS, 1).astype(np.float32))

    shared = dict(xn0=xn0h, xadd=xaddh, cosT=cosT, sinT=sinT, cosT2=cosT2,
                  sinT2=sinT2, perm=permh, m1=m1h, am2=am2h, lnw=lnwh,
                  w1t=w1h, w2t=w2h, b1c=b1c, gcol=gcol, bcol=bcol, b2c=b2c)

    per_layer = []
    for l in range(c.L):
        wqkv = dequant_np(inputs["qkv_packed"][l], inputs["qkv_absmax"][l],
                          3 * c.H, c.H)
        wo = dequant_np(inputs["o_packed"][l], inputs["o_absmax"][l],
                        c.H, c.H)
        wgu = dequant_np(inputs["gu_packed"][l], inputs["gu_absmax"][l],
                         2 * c.FF, c.H)
        wd = dequant_np(inputs["down_packed"][l], inputs["down_absmax"][l],
                        c.H, c.FF)
        per_layer.append((wqkv, wo, wgu, wd))

    in_maps = []
    for core in range(c.NC):
        m = dict(shared)
        d0 = core * c.DR
        g0 = core * c.FPC
        for l in range(c.L):
            wqkv, wo, wgu, wd = per_layer[l]
            m[f"wq{l}"] = _flat_kpm(wqkv[d0:d0 + c.DR, :].T, P)
            m[f"wk{l}"] = _flat_kpm(wqkv[c.H + d0:c.H + d0 + c.DR, :].T, P)
            m[f"wv{l}"] = _flat_kpm(wqkv[2 * c.H + d0:2 * c.H + d0 + c.DR, :].T, P)
            # o input-sharded: lhsT rows = this core's ctx feats
            woT = np.ascontiguousarray(wo.T[d0:d0 + c.DR, :])     # [DR, H]
            m[f"wo{l}"] = np.ascontiguousarray(
                woT.reshape(c.HPC, c.HD, c.KT, P).transpose(1, 2, 0, 3)
                .reshape(c.HD, c.KT * c.HPC * P).astype(BF16))
            m[f"wg{l}"] = _flat_kpm(wgu[g0:g0 + c.FPC, :].T, P)
            m[f"wu{l}"] = _flat_kpm(wgu[c.FF + g0:c.FF + g0 + c.FPC, :].T, P)
            # down input-sharded: lhsT rows = this core's ff feats
            wdT = np.ascontiguousarray(wd.T[g0:g0 + c.FPC, :])    # [FPC, H]
            m[f"wd{l}"] = np.ascontiguousarray(
                wdT.reshape(c.FT, P, c.KT, P).transpose(1, 2, 0, 3)
                .reshape(P, c.KT * c.FT * P).astype(BF16))
        in_maps.append(m)
    return in_maps


# ----------------------------------------------------------------------------
# device kernel
# ----------------------------------------------------------------------------

def build_nc(cfg: Cfg):
    import concourse.bass as bass
    import concourse.mybir as mybir
    import concourse.tile as tile
    from concourse import bacc

    c = cfg
    c.check()
    P = c.P
    f32 = mybir.dt.float32
    bf16 = mybir.dt.bfloat16
    f16 = mybir.dt.float16
    AF = mybir.ActivationFunctionType
    OP = mybir.AluOpType
    X = mybir.AxisListType.X

    nc = bacc.Bacc("TRN2", target_bir_lowering=False, debug=False,
                   enable_asserts=False, num_devices=c.NC)
    RG = [list(range(c.NC))]
    SHARED = "Shared" if c.NC > 4 else "Local"
    nln = 2 * c.L + 1

    def din(name, shape, dt):
        return nc.dram_tensor(name, list(shape), dt, kind="ExternalInput").ap()

    xn0_d = din("xn0", [P, c.KT * c.T], bf16)
    xadd_d = din("xadd", [P, c.KT * c.T], f16)
    cosT = din("cosT", [c.HD, c.T], f32)
    sinT = din("sinT", [c.HD, c.T], f32)
    cosT2 = din("cosT2", [c.HD, c.B], f32)
    sinT2 = din("sinT2", [c.HD, c.B], f32)
    perm_d = din("perm", [c.HD, c.HD], bf16)
    m1 = din("m1", [P, c.B * c.SP * c.S], bf16)
    am2 = din("am2", [P, c.B * c.SP], bf16)
    lnw_d = din("lnw", [P, nln * c.KT], f32)
    w1t = din("w1t", [P, c.KT * c.CLS], bf16)
    w2t = din("w2t", [P, c.CT * c.NCLS], bf16)
    b1c = din("b1c", [P, c.CT], f32)
    gcol = din("gcol", [P, c.CT], f32)
    bcol = din("bcol", [P, c.CT], f32)
    b2c = din("b2c", [c.NCLS, 1], f32)
    wq = [din(f"wq{l}", [P, c.KT * c.DR], bf16) for l in range(c.L)]
    wk = [din(f"wk{l}", [P, c.KT * c.DR], bf16) for l in range(c.L)]
    wv = [din(f"wv{l}", [P, c.KT * c.DR], bf16) for l in range(c.L)]
    wo = [din(f"wo{l}", [c.HD, c.KT * c.HPC * P], bf16) for l in range(c.L)]
    wg = [din(f"wg{l}", [P, c.KT * c.FPC], bf16) for l in range(c.L)]
    wu = [din(f"wu{l}", [P, c.KT * c.FPC], bf16) for l in range(c.L)]
    wd = [din(f"wd{l}", [P, c.KT * c.FT * P], bf16) for l in range(c.L)]
    out_d = nc.dram_tensor("logits_out", [c.NCLS, c.B], f32,
                           kind="ExternalOutput").ap()

    isqrt_hd = 1.0 / math.sqrt(c.HD)
    WCH = 4096  # weight stream chunk columns (bf16)

    with tile.TileContext(nc) as tc, ExitStack() as ctx:
        const = ctx.enter_context(tc.tile_pool(name="const", bufs=1))
        persist = ctx.enter_context(tc.tile_pool(name="persist", bufs=1))
        wpool = ctx.enter_context(tc.tile_pool(name="wpool", bufs=5))
        xpool = ctx.enter_context(tc.tile_pool(name="xpool", bufs=3))
        spool = ctx.enter_context(tc.tile_pool(name="spool", bufs=2))
        ppool = ctx.enter_context(tc.tile_pool(name="ppool", bufs=3))
        rpool = ctx.enter_context(tc.tile_pool(name="rpool", bufs=2))
        psum = ctx.enter_context(tc.tile_pool(name="psum", bufs=8,
                                              space="PSUM"))
        dram = ctx.enter_context(tc.tile_pool(name="dram", bufs=1,
                                              space="DRAM"))

        # ---- constants ----
        ones_cbf = const.tile([P, 1], bf16, tag="ones_cbf")
        nc.vector.memset(ones_cbf[:], 1.0)
        ones_c32 = const.tile([P, 1], f32, tag="ones_c32")
        nc.vector.memset(ones_c32[:], 1.0)
        ones_r32 = const.tile([1, P], f32, tag="ones_r32")
        nc.vector.memset(ones_r32[:], 1.0)
        ones_rbf = const.tile([1, P], bf16, tag="ones_rbf")
        nc.vector.memset(ones_rbf[:], 1.0)
        eps_col = const.tile([P, 1], f32, tag="eps_col")
        nc.vector.memset(eps_col[:], EPS)
        inv_nc = const.tile([P, 1], f32, tag="inv_nc")
        nc.vector.memset(inv_nc[:], 1.0 / c.NC)
        cos_sb = const.tile([c.HD, c.T], f32, tag="cos_sb")
        nc.sync.dma_start(out=cos_sb[:], in_=cosT)
        sin_sb = const.tile([c.HD, c.T], f32, tag="sin_sb")
        nc.sync.dma_start(out=sin_sb[:], in_=sinT)
        cos2_sb = const.tile([c.HD, c.B], f32, tag="cos2_sb")
        nc.sync.dma_start(out=cos2_sb[:], in_=cosT2)
        sin2_sb = const.tile([c.HD, c.B], f32, tag="sin2_sb")
        nc.sync.dma_start(out=sin2_sb[:], in_=sinT2)
        perm_sb = const.tile([c.HD, c.HD], bf16, tag="perm_sb")
        nc.sync.dma_start(out=perm_sb[:], in_=perm_d)
        mask_sb = const.tile([P, c.B * c.SP, c.S], bf16, tag="mask_sb")
        nc.sync.dma_start(
            out=mask_sb[:].rearrange("p a s -> p (a s)"), in_=m1)
        am2_sb = const.tile([P, c.B * c.SP], bf16, tag="am2_sb")
        nc.sync.dma_start(out=am2_sb[:], in_=am2)
        lnw_sb = const.tile([P, nln, c.KT], f32, tag="lnw_sb")
        nc.sync.dma_start(
            out=lnw_sb[:].rearrange("p a k -> p (a k)"), in_=lnw_d)
        b1_sb = const.tile([P, c.CT], f32, tag="b1_sb")
        nc.sync.dma_start(out=b1_sb[:], in_=b1c)
        g_sb = const.tile([P, c.CT], f32, tag="g_sb")
        nc.sync.dma_start(out=g_sb[:], in_=gcol)
        bcol_sb = const.tile([P, c.CT], f32, tag="bcol_sb")
        nc.sync.dma_start(out=bcol_sb[:], in_=bcol)
        b2_sb = const.tile([c.NCLS, 1], f32, tag="b2_sb")
        nc.sync.dma_start(out=b2_sb[:], in_=b2c)
        w2_sb = const.tile([P, c.CT * c.NCLS], bf16, tag="w2_sb")
        nc.sync.dma_start(out=w2_sb[:], in_=w2t)

        # ---- collective warm-up ----
        wu_sb = const.tile([P, 512], f32, tag="wu_sb")
        nc.vector.memset(wu_sb[:], 0.0)
        wu_in = dram.tile([P, 512], f32, tag="wu_in", name="wu_in")
        wu_out = dram.tile([P, 512], f32, addr_space=SHARED,
                           tag="wu_out", name="wu_out")
        nc.gpsimd.dma_start(out=wu_in[:], in_=wu_sb[:])
        nc.gpsimd.collective_compute(
            "AllReduce", OP.add, replica_groups=RG,
            ins=[wu_in[:]], outs=[wu_out[:]])

        # ---- persistent activations ----
        xn = persist.tile([P, c.KT, c.T], bf16, tag="xn")
        nc.sync.dma_start(
            out=xn[:].rearrange("p k t -> p (k t)"), in_=xn0_d)
        q_rot = persist.tile([c.HD, c.HPC, c.T], bf16, tag="qrot")
        k_rot = persist.tile([c.HD, c.HPC, c.T], bf16, tag="krot")
        v_sb = persist.tile([P, c.B * c.SP, c.DR], bf16, tag="vsb")
        ctx_sb = persist.tile([c.HD, c.HPC, c.T], bf16, tag="ctxsb")

        # AR dram buffers (fp16): [p, kt*S+s] per batch
        aro_in = [dram.tile([P, c.KT * c.S], f16, tag=f"aroi{b}",
                            name=f"aroi{b}") for b in range(c.B)]
        aro_out = [dram.tile([P, c.KT * c.S], f16, addr_space=SHARED,
                             tag=f"aroo{b}", name=f"aroo{b}")
                   for b in range(c.B)]
        ard_in = [dram.tile([P, c.KT * c.S], f16, tag=f"ardi{b}",
                            name=f"ardi{b}") for b in range(c.B)]
        ard_out = [dram.tile([P, c.KT * c.S], f16, addr_space=SHARED,
                             tag=f"ardo{b}", name=f"ardo{b}")
                   for b in range(c.B)]
        ar2o_in = dram.tile([P, c.KT * c.B], f16, tag="ar2oi", name="ar2oi")
        ar2o_out = dram.tile([P, c.KT * c.B], f16, addr_space=SHARED,
                             tag="ar2oo", name="ar2oo")
        ar2d_in = dram.tile([P, c.KT * c.B], f16, tag="ar2di", name="ar2di")
        ar2d_out = dram.tile([P, c.KT * c.B], f16, addr_space=SHARED,
                             tag="ar2do", name="ar2do")

        # ---------- helpers ----------
        def wchunks(units, cpu):
            """Split `units` units of cpu columns into <=WCH-col chunks."""
            cap = max(1, WCH // cpu)
            out = []
            u0 = 0
            while u0 < units:
                un = min(cap, units - u0)
                out.append((u0, un))
                u0 += un
            return out

        def stream(wsrc, units, cpu, rows, fn, name):
            """Stream flat weight [rows, units*cpu] in chunks; call
            fn(u, wt, col0) per unit."""
            for ci, (u0, un) in enumerate(wchunks(units, cpu)):
                wt = wpool.tile([P, WCH], bf16, tag="wt", name=f"{name}{ci}")
                nc.sync.dma_start(
                    out=wt[0:rows, 0:un * cpu],
                    in_=wsrc[:, u0 * cpu:(u0 + un) * cpu])
                for u in range(u0, u0 + un):
                    fn(u, wt, (u - u0) * cpu)

        def emit_rope(src_ps, dst, cos_ap, sin_ap, ncols, pname):
            """dst = src*cos + perm(src)*sin  (sin sign-folded)."""
            qs = rpool.tile([c.HD, ncols], bf16, tag="qs", name=f"qs{pname}")
            nc.scalar.copy(qs[:], src_ps[:])
            rps = psum.tile([c.HD, ncols], f32, tag="ps", name=f"rp{pname}")
            nc.tensor.matmul(rps[:], perm_sb[:], qs[:], start=True, stop=True)
            t1 = rpool.tile([c.HD, ncols], f32, tag="t1", name=f"t1{pname}")
            nc.vector.tensor_mul(t1[:], qs[:], cos_ap)
            t2 = rpool.tile([c.HD, ncols], f32, tag="t2", name=f"t2{pname}")
            nc.vector.tensor_mul(t2[:], rps[:], sin_ap)
            nc.vector.tensor_add(dst, t1[:], t2[:])

        def emit_norm_full(src_flat, lnidx, b, name):
            """x tiles from dram [P, KT*S] fp16 -> xn[:, :, b-cols] (bf16)."""
            c0 = b * c.S
            ss = psum.tile([1, c.S], f32, tag="ps", name=f"ss{name}")
            xfs = []
            for kt in range(c.KT):
                xf = xpool.tile([P, c.S], f16, tag="xf", name=f"xf{name}",
                                bufs=3)
                nc.scalar.dma_start(
                    out=xf[:], in_=src_flat[:, kt * c.S:(kt + 1) * c.S])
                sq = xpool.tile([P, c.S], bf16, tag="sq", name=f"sq{name}",
                                bufs=2)
                nc.scalar.activation(sq[:], xf[:], AF.Square)
                nc.tensor.matmul(ss[:], ones_cbf[:], sq[:],
                                 start=(kt == 0), stop=(kt == c.KT - 1))
                nc.vector.tensor_copy(xn[:, kt, c0:c0 + c.S], xf[:])
            lt = spool.tile([1, c.S], f32, tag="lt", name=f"lt{name}")
            nc.scalar.activation(lt[:], ss[:], AF.Ln,
                                 bias=eps_col[0:1, :], scale=1.0 / c.H)
            rt = spool.tile([1, c.S], f32, tag="rt", name=f"rt{name}")
            nc.scalar.activation(rt[:], lt[:], AF.Exp, scale=-0.5)
            bb = psum.tile([P, c.S], f32, tag="ps", name=f"bb{name}")
            nc.tensor.matmul(bb[:], ones_r32[:], rt[:], start=True, stop=True)
            bc = spool.tile([P, c.S], f32, tag="bc", name=f"bc{name}")
            nc.scalar.copy(bc[:], bb[:])
            for kt in range(c.KT):
                nc.vector.scalar_tensor_tensor(
                    xn[:, kt, c0:c0 + c.S], xn[:, kt, c0:c0 + c.S],
                    lnw_sb[:, lnidx, kt:kt + 1], bc[:], OP.mult, OP.mult)

        def emit_norm_slim(src_sb, lnidx, dst, name):
            """src_sb [P, KT, B] fp16 -> dst [P, KT, B] bf16."""
            sq = spool.tile([P, c.KT, c.B], bf16, tag="sqs", name=f"sqs{name}")
            nc.scalar.activation(
                sq[:].rearrange("p k b -> p (k b)"),
                src_sb[:].rearrange("p k b -> p (k b)"), AF.Square)
            sp_ = psum.tile([1, c.KT * c.B], f32, tag="ps", name=f"sp{name}")
            nc.tensor.matmul(sp_[:], ones_cbf[:],
                             sq[:].rearrange("p k b -> p (k b)"),
                             start=True, stop=True)
            ss2 = spool.tile([1, c.B], f32, tag="ss2", name=f"ss2{name}")
            nc.vector.tensor_reduce(
                ss2[:], sp_[:].rearrange("o (k b) -> o b k", b=c.B), X, OP.add)
            lt = spool.tile([1, c.B], f32, tag="lts", name=f"lts{name}")
            nc.scalar.activation(lt[:], ss2[:], AF.Ln,
                                 bias=eps_col[0:1, :], scale=1.0 / c.H)
            rt = spool.tile([1, c.B], f32, tag="rts", name=f"rts{name}")
            nc.scalar.activation(rt[:], lt[:], AF.Exp, scale=-0.5)
            bb = psum.tile([P, c.B], f32, tag="ps", name=f"bbs{name}")
            nc.tensor.matmul(bb[:], ones_r32[:], rt[:], start=True, stop=True)
            bc = spool.tile([P, c.B], f32, tag="bcs", name=f"bcs{name}")
            nc.scalar.copy(bc[:], bb[:])
            for kt in range(c.KT):
                nc.vector.scalar_tensor_tensor(
                    dst[:, kt, :], src_sb[:, kt, :],
                    lnw_sb[:, lnidx, kt:kt + 1], bc[:], OP.mult, OP.mult)

        def qk_pass(wsrc, rot_dst, cos_ap2, sin_ap2, l, name):
            """q or k projection for all batches + rope."""
            ps = {}
            for h in range(c.HPC):
                for b in range(c.B):
                    ps[(h, b)] = psum.tile([c.HD, c.S], f32, tag="ps",
                                           name=f"{name}p{h}_{b}")

            def fn(kt, wt, col0):
                for h in range(c.HPC):
                    for b in range(c.B):
                        nc.tensor.matmul(
                            ps[(h, b)][:],
                            wt[:, col0 + h * c.HD:col0 + (h + 1) * c.HD],
                            xn[:, kt, b * c.S:(b + 1) * c.S],
                            start=(kt == 0), stop=(kt == c.KT - 1))
            stream(wsrc, c.KT, c.DR, P, fn, name)
            for h in range(c.HPC):
                for b in range(c.B):
                    cs = b * c.S
                    emit_rope(ps[(h, b)], rot_dst[:, h, cs:cs + c.S],
                              cos_ap2[:, cs:cs + c.S], sin_ap2[:, cs:cs + c.S],
                              c.S, f"{name}{h}{b}")

        def v_pass(l, name):
            ps = {}
            for b in range(c.B):
                for t in range(c.SP):
                    ps[(b, t)] = psum.tile([P, c.DR], f32, tag="ps",
                                           name=f"{name}p{b}_{t}")

            def fn(kt, wt, col0):
                for b in range(c.B):
                    for t in range(c.SP):
                        tc0 = b * c.S + t * P
                        nc.tensor.matmul(
                            ps[(b, t)][:], xn[:, kt, tc0:tc0 + P],
                            wt[:, col0:col0 + c.DR],
                            start=(kt == 0), stop=(kt == c.KT - 1))
            stream(wv[l], c.KT, c.DR, P, fn, name)
            for b in range(c.B):
                for t in range(c.SP):
                    nc.scalar.copy(v_sb[:, b * c.SP + t, :], ps[(b, t)][:])

        def attn_full(b, l):
            """attention for batch b -> ctx_sb[:, :, b-cols]."""
            cs = b * c.S
            for h in range(c.HPC):
                den = psum.tile([1, c.S], f32, tag="ps", name=f"den{l}{b}{h}")
                cps = psum.tile([c.HD, c.S], f32, tag="ps",
                                name=f"cps{l}{b}{h}")
                for t in range(c.SP):
                    sps = psum.tile([P, c.S], f32, tag="ps",
                                    name=f"sps{l}{b}{h}{t}")
                    nc.tensor.matmul(
                        sps[:], k_rot[:, h, cs + t * P:cs + (t + 1) * P],
                        q_rot[:, h, cs:cs + c.S], start=True, stop=True)
                    pt = ppool.tile([P, c.S], bf16, tag="pt", name="pt")
                    nc.scalar.activation(pt[:], sps[:], AF.Exp,
                                         scale=isqrt_hd)
                    nc.vector.tensor_mul(
                        pt[:], pt[:], mask_sb[:, b * c.SP + t, :])
                    nc.tensor.matmul(den[:], ones_cbf[:], pt[:],
                                     start=(t == 0), stop=(t == c.SP - 1))
                    nc.tensor.matmul(
                        cps[:], v_sb[:, b * c.SP + t, h * c.HD:(h + 1) * c.HD],
                        pt[:], start=(t == 0), stop=(t == c.SP - 1))
                dr = spool.tile([1, c.S], f32, tag="dr", name="dr")
                nc.vector.reciprocal(dr[:], den[:])
                bb = psum.tile([c.HD, c.S], f32, tag="ps", name=f"ab{l}{b}{h}")
                nc.tensor.matmul(bb[:], ones_r32[:, 0:c.HD], dr[:],
                                 start=True, stop=True)
                bsb = spool.tile([c.HD, c.S], f32, tag="bsb", name="bsb")
                nc.vector.tensor_copy(bsb[:], bb[:])
                nc.vector.tensor_mul(ctx_sb[:, h, cs:cs + c.S], cps[:],
                                     bsb[:])

        def o_pass(b, l):
            """o projection partial for batch b + x/NC -> aro_in[b]."""
            cs = b * c.S

            def fn(ot, wt, col0):
                ps = psum.tile([P, c.S], f32, tag="ps", name=f"op{l}{b}{ot}")
                for h in range(c.HPC):
                    nc.tensor.matmul(
                        ps[:], wt[0:c.HD, col0 + h * P:col0 + (h + 1) * P],
                        ctx_sb[:, h, cs:cs + c.S],
                        start=(h == 0), stop=(h == c.HPC - 1))
                xa = xpool.tile([P, c.S], f16, tag="xa", name="xa", bufs=3)
                nc.scalar.dma_start(
                    out=xa[:],
                    in_=xadd_d[:, ot * c.T + cs:ot * c.T + cs + c.S])
                st = xpool.tile([P, c.S], f16, tag="st", name="st", bufs=3)
                nc.vector.tensor_add(st[:], xa[:], ps[:])
                nc.gpsimd.dma_start(
                    out=aro_in[b][:, ot * c.S:(ot + 1) * c.S], in_=st[:])
            stream(wo[l], c.KT, c.HPC * P, c.HD, fn, f"o{l}{b}")

        def gu_pass(b, l, gact_t):
            """gate/up for batch b -> gact_t [P, FT, S]."""
            cs = b * c.S
            gps = [psum.tile([P, c.S], f32, tag="ps", name=f"g{l}{b}{ot}")
                   for ot in range(c.FT)]

            def gfn(kt, wt, col0):
                for ot in range(c.FT):
                    nc.tensor.matmul(
                        gps[ot][:], wt[:, col0 + ot * P:col0 + (ot + 1) * P],
                        xn[:, kt, cs:cs + c.S],
                        start=(kt == 0), stop=(kt == c.KT - 1))
            stream(wg[l], c.KT, c.FPC, P, gfn, f"g{l}{b}")
            for ot in range(c.FT):
                sgt = xpool.tile([P, c.S], bf16, tag="sgt", name="sgt",
                                 bufs=2)
                nc.scalar.activation(sgt[:], gps[ot][:], AF.Sigmoid)
                nc.vector.tensor_mul(gact_t[:, ot, :], gps[ot][:], sgt[:])
            ups = [psum.tile([P, c.S], f32, tag="ps", name=f"u{l}{b}{ot}")
                   for ot in range(c.FT)]

            def ufn(kt, wt, col0):
                for ot in range(c.FT):
                    nc.tensor.matmul(
                        ups[ot][:], wt[:, col0 + ot * P:col0 + (ot + 1) * P],
                        xn[:, kt, cs:cs + c.S],
                        start=(kt == 0), stop=(kt == c.KT - 1))
            stream(wu[l], c.KT, c.FPC, P, ufn, f"u{l}{b}")
            for ot in range(c.FT):
                nc.vector.tensor_mul(gact_t[:, ot, :], gact_t[:, ot, :],
                                     ups[ot][:])

        def down_pass(b, l, gact_t):
            """down partial for batch b + x/NC -> ard_in[b]."""
            def fn(ot, wt, col0):
                ps = psum.tile([P, c.S], f32, tag="ps", name=f"dp{l}{b}{ot}")
                for kt in range(c.FT):
                    nc.tensor.matmul(
                        ps[:], wt[:, col0 + kt * P:col0 + (kt + 1) * P],
                        gact_t[:, kt, :],
                        start=(kt == 0), stop=(kt == c.FT - 1))
                x1 = xpool.tile([P, c.S], f16, tag="x1", name="x1", bufs=3)
                nc.scalar.dma_start(
                    out=x1[:], in_=aro_out[b][:, ot * c.S:(ot + 1) * c.S])
                st = xpool.tile([P, c.S], f16, tag="st", name="st2", bufs=3)
                nc.vector.scalar_tensor_tensor(
                    st[:], x1[:], inv_nc[:], ps[:], OP.mult, OP.add)
                nc.gpsimd.dma_start(
                    out=ard_in[b][:, ot * c.S:(ot + 1) * c.S], in_=st[:])
            stream(wd[l], c.KT, c.FT * P, P, fn, f"d{l}{b}")

        def allreduce(inb, outb, name):
            nc.gpsimd.collective_compute(
                "AllReduce", OP.add, replica_groups=RG,
                ins=[inb[:]], outs=[outb[:]])

        # ================= layer 0 (full) =================
        l = 0
        qk_pass(wq[l], q_rot, cos_sb, sin_sb, l, f"q{l}")
        qk_pass(wk[l], k_rot, cos_sb, sin_sb, l, f"k{l}")
        v_pass(l, f"v{l}")

        attn_full(0, l)
        o_pass(0, l)
        allreduce(aro_in[0], aro_out[0], f"aro{l}0")
        attn_full(1, l)
        o_pass(1, l)
        allreduce(aro_in[1], aro_out[1], f"aro{l}1")

        gact0 = persist.tile([P, c.FT, c.S], bf16, tag="gact", name="gact0",
                             bufs=2)
        emit_norm_full(aro_out[0], 2 * l + 1, 0, f"ln2_{l}0")
        gu_pass(0, l, gact0)
        down_pass(0, l, gact0)
        allreduce(ard_in[0], ard_out[0], f"ard{l}0")
        gact1 = persist.tile([P, c.FT, c.S], bf16, tag="gact", name="gact1",
                             bufs=2)
        emit_norm_full(aro_out[1], 2 * l + 1, 1, f"ln2_{l}1")
        gu_pass(1, l, gact1)
        down_pass(1, l, gact1)
        allreduce(ard_in[1], ard_out[1], f"ard{l}1")

        # ================= layer 1 (slim last layer) =================
        l = 1
        emit_norm_full(ard_out[0], 2 * l, 0, f"ln1_{l}0")
        emit_norm_full(ard_out[1], 2 * l, 1, f"ln1_{l}1")
        qk_pass(wk[l], k_rot, cos_sb, sin_sb, l, f"k{l}")
        v_pass(l, f"v{l}")

        # q at last tokens only
        xnl = xn[:].rearrange("p k (b s) -> p k b s", s=c.S)[:, :, :, c.S - 1]
        q2ps = [psum.tile([c.HD, c.B], f32, tag="ps", name=f"q2p{h}")
                for h in range(c.HPC)]

        def q2fn(kt, wt, col0):
            for h in range(c.HPC):
                nc.tensor.matmul(
                    q2ps[h][:], wt[:, col0 + h * c.HD:col0 + (h + 1) * c.HD],
                    xnl[:, kt, :], start=(kt == 0), stop=(kt == c.KT - 1))
        stream(wq[l], c.KT, c.DR, P, q2fn, "q2")
        q2_sb = persist.tile([c.HD, c.HPC, c.B], bf16, tag="q2sb")
        for h in range(c.HPC):
            emit_rope(q2ps[h], q2_sb[:, h, :], cos2_sb[:], sin2_sb[:],
                      c.B, f"q2r{h}")

        # x2 at last tokens (fp16) for the slim residual carries
        x2last = persist.tile([P, c.KT, c.B], f16, tag="x2last")
        for b in range(c.B):
            nc.scalar.dma_start(
                out=x2last[:, :, b],
                in_=ard_out[b][:].rearrange("p (k s) -> p k s",
                                            s=c.S)[:, :, c.S - 1])

        # slim attention -> ctx2 [HD, HPC, B]
        ctx2_sb = persist.tile([c.HD, c.HPC, c.B], bf16, tag="ctx2sb")
        for b in range(c.B):
            for h in range(c.HPC):
                sps = psum.tile([P, c.SP], f32, tag="ps", name=f"s2{b}{h}")
                for t in range(c.SP):
                    nc.tensor.matmul(
                        sps[:, t:t + 1],
                        k_rot[:, h, b * c.S + t * P:b * c.S + (t + 1) * P],
                        q2_sb[:, h, b:b + 1], start=True, stop=True)
                pt = ppool.tile([P, c.SP], bf16, tag="pt2", name="pt2")
                nc.scalar.activation(pt[:], sps[:], AF.Exp, scale=isqrt_hd)
                nc.vector.tensor_mul(
                    pt[:], pt[:], am2_sb[:, b * c.SP:(b + 1) * c.SP])
                dps = psum.tile([1, c.SP], f32, tag="ps", name=f"d2{b}{h}")
                nc.tensor.matmul(dps[:], ones_cbf[:], pt[:],
                                 start=True, stop=True)
                d1 = spool.tile([1, 1], f32, tag="d1", name="d1")
                nc.vector.tensor_reduce(d1[:], dps[:], X, OP.add)
                r1 = spool.tile([1, 1], f32, tag="r1", name="r1")
                nc.vector.reciprocal(r1[:], d1[:])
                cps = psum.tile([c.HD, 1], f32, tag="ps", name=f"c2{b}{h}")
                for t in range(c.SP):
                    nc.tensor.matmul(
                        cps[:],
                        v_sb[:, b * c.SP + t, h * c.HD:(h + 1) * c.HD],
                        pt[:, t:t + 1], start=(t == 0), stop=(t == c.SP - 1))
                bb = psum.tile([c.HD, 1], f32, tag="ps", name=f"b2{b}{h}")
                nc.tensor.matmul(bb[:], ones_r32[:, 0:c.HD], r1[:],
                                 start=True, stop=True)
                bsb = spool.tile([c.HD, 1], f32, tag="bsb2", name="bsb2")
                nc.vector.tensor_copy(bsb[:], bb[:])
                nc.vector.tensor_mul(ctx2_sb[:, h, b:b + 1], cps[:], bsb[:])

        # slim o projection (+ x2/NC) -> one AR for both batches
        def o2fn(ot, wt, col0):
            ps = psum.tile([P, c.B], f32, tag="ps", name=f"o2p{ot}")
            for h in range(c.HPC):
                nc.tensor.matmul(
                    ps[:], wt[0:c.HD, col0 + h * P:col0 + (h + 1) * P],
                    ctx2_sb[:, h, :], start=(h == 0), stop=(h == c.HPC - 1))
            st = xpool.tile([P, c.B], f16, tag="st3", name="st3", bufs=3)
            nc.vector.scalar_tensor_tensor(
                st[:], x2last[:, ot, :], inv_nc[:], ps[:], OP.mult, OP.add)
            nc.gpsimd.dma_start(
                out=ar2o_in[:, ot * c.B:(ot + 1) * c.B], in_=st[:])
        stream(wo[l], c.KT, c.HPC * P, c.HD, o2fn, "o2")
        allreduce(ar2o_in, ar2o_out, "ar2o")

        # slim MLP
        x3_sb = persist.tile([P, c.KT, c.B], f16, tag="x3sb")
        nc.scalar.dma_start(
            out=x3_sb[:].rearrange("p k b -> p (k b)"), in_=ar2o_out[:])
        xn2l = persist.tile([P, c.KT, c.B], bf16, tag="xn2l")
        emit_norm_slim(x3_sb, 2 * l + 1, xn2l, "ln2s")

        g2ps = [psum.tile([P, c.B], f32, tag="ps", name=f"g2p{ot}")
                for ot in range(c.FT)]

        def g2fn(kt, wt, col0):
            for ot in range(c.FT):
                nc.tensor.matmul(
                    g2ps[ot][:], wt[:, col0 + ot * P:col0 + (ot + 1) * P],
                    xn2l[:, kt, :], start=(kt == 0), stop=(kt == c.KT - 1))
        stream(wg[l], c.KT, c.FPC, P, g2fn, "g2")
        int2 = persist.tile([P, c.FT, c.B], bf16, tag="int2")
        for ot in range(c.FT):
            sg2 = spool.tile([P, c.B], bf16, tag="sg2", name="sg2")
            nc.scalar.activation(sg2[:], g2ps[ot][:], AF.Sigmoid)
            nc.vector.tensor_mul(int2[:, ot, :], g2ps[ot][:], sg2[:])
        u2ps = [psum.tile([P, c.B], f32, tag="ps", name=f"u2p{ot}")
                for ot in range(c.FT)]

        def u2fn(kt, wt, col0):
            for ot in range(c.FT):
                nc.tensor.matmul(
                    u2ps[ot][:], wt[:, col0 + ot * P:col0 + (ot + 1) * P],
                    xn2l[:, kt, :], start=(kt == 0), stop=(kt == c.KT - 1))
        stream(wu[l], c.KT, c.FPC, P, u2fn, "u2")
        for ot in range(c.FT):
            nc.vector.tensor_mul(int2[:, ot, :], int2[:, ot, :], u2ps[ot][:])

        def d2fn(ot, wt, col0):
            ps = psum.tile([P, c.B], f32, tag="ps", name=f"d2p{ot}")
            for kt in range(c.FT):
                nc.tensor.matmul(
                    ps[:], wt[:, col0 + kt * P:col0 + (kt + 1) * P],
                    int2[:, kt, :], start=(kt == 0), stop=(kt == c.FT - 1))
            st = xpool.tile([P, c.B], f16, tag="st4", name="st4", bufs=3)
            nc.vector.scalar_tensor_tensor(
                st[:], x3_sb[:, ot, :], inv_nc[:], ps[:], OP.mult, OP.add)
            nc.gpsimd.dma_start(
                out=ar2d_in[:, ot * c.B:(ot + 1) * c.B], in_=st[:])
        stream(wd[l], c.KT, c.FT * P, P, d2fn, "d2")
        allreduce(ar2d_in, ar2d_out, "ar2d")

        # ================= final norm + cls head =================
        x4_sb = persist.tile([P, c.KT, c.B], f16, tag="x4sb")
        nc.scalar.dma_start(
            out=x4_sb[:].rearrange("p k b -> p (k b)"), in_=ar2d_out[:])
        xnf = persist.tile([P, c.KT, c.B], bf16, tag="xnf")
        emit_norm_slim(x4_sb, 2 * c.L, xnf, "lnf")

        hps = [psum.tile([P, c.B], f32, tag="ps", name=f"hps{ct}")
               for ct in range(c.CT)]

        def w1fn(kt, wt, col0):
            for ct in range(c.CT):
                nc.tensor.matmul(
                    hps[ct][:], wt[:, col0 + ct * P:col0 + (ct + 1) * P],
                    xnf[:, kt, :], start=(kt == 0), stop=(kt == c.KT - 1))
        stream(w1t, c.KT, c.CLS, P, w1fn, "w1")

        h_sb = persist.tile([P, c.CT, c.B], bf16, tag="h_sb")
        mn = psum.tile([1, c.B], f32, tag="ps", name="mn")
        ssq = psum.tile([1, c.B], f32, tag="ps", name="ssq")
        for ct in range(c.CT):
            nc.scalar.activation(h_sb[:, ct, :], hps[ct][:], AF.Relu,
                                 bias=b1_sb[:, ct:ct + 1])
            hq = spool.tile([P, c.B], f32, tag="hq", name="hq")
            nc.vector.tensor_mul(hq[:], h_sb[:, ct, :], h_sb[:, ct, :])
            nc.tensor.matmul(mn[:], ones_cbf[:], h_sb[:, ct, :],
                             start=(ct == 0), stop=(ct == c.CT - 1))
            nc.tensor.matmul(ssq[:], ones_c32[:], hq[:],
                             start=(ct == 0), stop=(ct == c.CT - 1))
        m_sb = spool.tile([1, c.B], f32, tag="m_sb", name="m_sb")
        nc.vector.tensor_scalar_mul(m_sb[:], mn[:], 1.0 / c.CLS)
        s_sb = spool.tile([1, c.B], f32, tag="s_sb", name="s_sb")
        nc.vector.tensor_scalar_mul(s_sb[:], ssq[:], 1.0 / c.CLS)
        msq = spool.tile([1, c.B], f32, tag="msq", name="msq")
        nc.vector.tensor_mul(msq[:], m_sb[:], m_sb[:])
        var = spool.tile([1, c.B], f32, tag="var", name="var")
        nc.vector.tensor_sub(var[:], s_sb[:], msq[:])
        lv = spool.tile([1, c.B], f32, tag="lv", name="lv")
        nc.scalar.activation(lv[:], var[:], AF.Ln, bias=eps_col[0:1, :])
        rstd = spool.tile([1, c.B], f32, tag="rstd", name="rstd")
        nc.scalar.activation(rstd[:], lv[:], AF.Exp, scale=-0.5)
        bmp = psum.tile([P, c.B], f32, tag="ps", name="bmp")
        nc.tensor.matmul(bmp[:], ones_r32[:], m_sb[:], start=True, stop=True)
        bm_sb = spool.tile([P, c.B], f32, tag="bm", name="bm")
        nc.vector.tensor_copy(bm_sb[:], bmp[:])
        brp = psum.tile([P, c.B], f32, tag="ps", name="brp")
        nc.tensor.matmul(brp[:], ones_r32[:], rstd[:], start=True, stop=True)
        br_sb = spool.tile([P, c.B], f32, tag="br", name="br")
        nc.vector.tensor_copy(br_sb[:], brp[:])

        lg = psum.tile([c.NCLS, c.B], f32, tag="ps", name="lg")
        for ct in range(c.CT):
            t1 = spool.tile([P, c.B], f32, tag="ct1", name="ct1")
            nc.vector.tensor_sub(t1[:], h_sb[:, ct, :], bm_sb[:])
            t2 = spool.tile([P, c.B], f32, tag="ct2", name="ct2")
            nc.vector.tensor_mul(t2[:], t1[:], br_sb[:])
            hn = spool.tile([P, c.B], bf16, tag="hn", name="hn")
            nc.vector.tensor_scalar(hn[:], t2[:], g_sb[:, ct:ct + 1],
                                    bcol_sb[:, ct:ct + 1], OP.mult, OP.add)
            nc.tensor.matmul(
                lg[:], w2_sb[:, ct * c.NCLS:(ct + 1) * c.NCLS], hn[:],
                start=(ct == 0), stop=(ct == c.CT - 1))
        lg_sb = spool.tile([c.NCLS, c.B], f32, tag="lg_sb", name="lg_sb")
        nc.vector.tensor_scalar(lg_sb[:], lg[:], b2_sb[:], None, OP.add)
        nc.sync.dma_start(out=out_d, in_=lg_sb[:])

    nc.compile()
    return nc


# ----------------------------------------------------------------------------
# entry point
# ----------------------------------------------------------------------------

_CACHE = {}


def _get_nc(cfg):
    if cfg not in _CACHE:
        _CACHE[cfg] = build_nc(cfg)
    return _CACHE[cfg]


def run(cfg, inputs, trace=False, **kw):
    from concourse.bass_utils import run_bass_kernel_spmd
    in_maps = host_prep(cfg, inputs)
    nc = _get_nc(cfg)
    res = run_bass_kernel_spmd(nc, in_maps, core_ids=list(range(cfg.NC)),
                               trace=trace, **kw)
    out = np.asarray(res.results[0]["logits_out"])  # [NCLS, B]
    return np.ascontiguousarray(out.T.astype(np.float32)), res


def kernel(**inputs):
    inputs = {k: np.asarray(v) for k, v in inputs.items()}
    out, _ = run(FULL_CFG, inputs)
    return out
